# revision 51
# baseline (speedup 1.0000x reference)
"""Trainium2 Bass kernel for nn_CNL_5 (5-scale context non-local block).

Sharding: 8 cores = 4 samples x 2 query-subsets (n mod 36 in [18h,18h+18)).
Each core owns a contiguous pixel-block of the torch no-permute reshape, so
the z_w projection + BN + scale-sum is local; BN batch stats travel as 65x65
Gram matrices through one small AllReduce.

v3: host-packed inputs (few large DMAs); attention in 384-wide query chunks
so every PSUM tile is one bank (sc ring 4 deep, 3 live op accumulators);
software-pipelined sc -> exp -> op with lag-2 op emission; next-scale p/g
convs interleaved into the attention k-loop; PSUM-direct softmax finish
(fast reciprocal); per-scale X tiles (no W restacking DMAs); vectorized BN
post-stats.
"""
import numpy as np
import ml_dtypes
from collections import deque
from contextlib import ExitStack

import concourse.bass as bass
import concourse.bacc as bacc
import concourse.tile as tile
from concourse import mybir
from concourse import bass_utils
from concourse.alu_op_type import AluOpType

F32 = mybir.dt.float32
F32R = mybir.dt.float32r
BF16 = mybir.dt.bfloat16
AFT = mybir.ActivationFunctionType

NCORES = 8
CP = 256
QL = 1152            # local queries per core
QC = 384             # query chunk (1 PSUM bank in f32)
NQC = QL // QC       # 3
QCH = [(0, 512), (512, 512), (1024, 128)]
CR = [64, 256, 512, 1024, 2048]
MS = [2304, 2304, 576, 144, 36]
CSH = [0.0, 10.0, 15.0, 25.0, 40.0]   # per-scale softmax shift constants
PCH = {0: [384] * 6, 1: [384] * 6, 2: [288, 288], 3: [144], 4: [36]}
EPS = 1e-5
NPIX = 4 * 2304.0    # BN count (global)

# resp packing: per (scale, c-chunk) a [cw, M_s] block at column rcol.
RCH = {}
_c = 0
for _s in range(5):
    RCH[_s] = []
    for _co in range(0, CR[_s], 128):
        _w = min(128, CR[_s] - _co)
        RCH[_s].append((_c, _co, _w))
        _c += MS[_s]
RESP_COLS = _c                      # 10944
P_OFF = [0, 2304, 4608, 5184, 5328]  # p_all column offset per scale
# global g-block index per (s, m-chunk)
GK0 = [0, 18, 36, 41, 43]
NGBLK = 44
# global C-chunk index per scale (pwt/gwt column blocks)
CK0 = [0, 1, 3, 7, 15]
TWT16_C = 64 * 31                   # twt (bf16) columns inside gwb
PWT16_C = TWT16_C + 128             # scale-0 p weights (bf16) inside gwb

# wbr (f32r) columns: pwt chunks then twt halves
PWT_C = 0
TWT_C = 64 * 31                     # 1984
WBR_COLS = TWT_C + 128              # 2112
# wbf (f32) columns
ZWT_C = 0
ZW5_C = 256
GAM_C = ZW5_C + 650                 # 906
BET_C = GAM_C + 10                  # 916
I_C = BET_C + 2                     # 918  identity [128,128]
SEL_C = I_C + 128                   # 1046 one-hot row selectors [10, 640]
WBF_COLS = SEL_C + 640              # 1686

_CACHED = {}
DEBUG = False


def mtiles(M):
    out, off = [], 0
    while off < M:
        w = min(128, M - off)
        out.append((off, w))
        off += w
    return out


def build():
    nc = bacc.Bacc("TRN2", target_bir_lowering=False, debug=False,
                   num_devices=NCORES)
    persp_d = nc.dram_tensor("persp", [CP, QL], BF16, kind="ExternalInput").ap()
    respf_d = nc.dram_tensor("respf", [128, RESP_COLS], F32R,
                             kind="ExternalInput").ap()
    resp16_d = nc.dram_tensor("resp16", [128, RESP_COLS], BF16,
                              kind="ExternalInput").ap()
    wbr_d = nc.dram_tensor("wbr", [128, WBR_COLS], F32R,
                           kind="ExternalInput").ap()
    wbf_d = nc.dram_tensor("wbf", [128, WBF_COLS], F32,
                           kind="ExternalInput").ap()
    gwb_d = nc.dram_tensor("gwb", [128, 64 * 31 + 192], BF16,
                           kind="ExternalInput").ap()
    out_d = nc.dram_tensor("out", [CP, QL], BF16, kind="ExternalOutput").ap()
    if DEBUG:
        dbgo_d = nc.dram_tensor("dbgo", [64, QL], BF16,
                                kind="ExternalOutput").ap()
        dbgd_d = nc.dram_tensor("dbgd", [128, QL], F32,
                                kind="ExternalOutput").ap()
        dbgx_d = nc.dram_tensor("dbgx", [320, QL], BF16,
                                kind="ExternalOutput").ap()
        dbgp_d = nc.dram_tensor("dbgp", [64, 5364], F32R,
                                kind="ExternalOutput").ap()
        dbgt_d = nc.dram_tensor("dbgt", [64, QL], F32R,
                                kind="ExternalOutput").ap()
        dbgg_d = nc.dram_tensor("dbgg", [128, 128 * NGBLK], BF16,
                                kind="ExternalOutput").ap()
        dbgs_d = nc.dram_tensor("dbgs", [128, 400], F32,
                                kind="ExternalOutput").ap()

    with tile.TileContext(nc) as tc, ExitStack() as ctx:
        sb = ctx.enter_context(tc.tile_pool(name="sb", bufs=1))
        sba = ctx.enter_context(tc.tile_pool(name="sba", bufs=2))
        dram = ctx.enter_context(tc.tile_pool(name="dram", bufs=1, space="DRAM"))
        psA = ctx.enter_context(tc.tile_pool(name="psA", bufs=4, space="PSUM"))
        psB = ctx.enter_context(tc.tile_pool(name="psB", bufs=4, space="PSUM"))

        # ---------------- input loads (few, large) ----------------
        wbr_sb = sb.tile([128, WBR_COLS], F32R, tag="wbr", name="wbr_sb")
        respf_sb = sb.tile([128, RESP_COLS], F32R, tag="respf", name="respf_sb")
        resp16_sb = sb.tile([128, RESP_COLS], BF16, tag="resp16", name="resp16_sb")
        gwb_sb = sb.tile([128, 64 * 31 + 192], BF16, tag="gwb", name="gwb_sb")
        # persp + s0 inputs first so t/p0/g0 convs start early
        persp_sb = [sb.tile([128, QL], BF16, tag=f"persp{t}", name=f"persp{t}")
                    for t in range(2)]
        nc.sync.dma_start(gwb_sb[:, TWT16_C:], gwb_d[:, TWT16_C:])
        for t in range(2):
            nc.sync.dma_start(persp_sb[t][:], persp_d[128 * t:128 * t + 128, :])
        nc.sync.dma_start(resp16_sb[:, 0:1152], resp16_d[:, 0:1152])
        nc.sync.dma_start(gwb_sb[:, 0:64], gwb_d[:, 0:64])
        nc.sync.dma_start(resp16_sb[:, 1152:2304], resp16_d[:, 1152:2304])
        nc.sync.dma_start(wbr_sb[:, 64:TWT_C], wbr_d[:, 64:TWT_C])
        nc.sync.dma_start(gwb_sb[:, 64:TWT16_C], gwb_d[:, 64:TWT16_C])
        nc.sync.dma_start(respf_sb[:, 2304:6912], respf_d[:, 2304:6912])
        nc.sync.dma_start(resp16_sb[:, 2304:6912], resp16_d[:, 2304:6912])
        wbf_sb = sb.tile([128, WBF_COLS], F32, tag="wbf", name="wbf_sb")
        nc.sync.dma_start(wbf_sb[:], wbf_d)
        nc.sync.dma_start(respf_sb[:, 6912:RESP_COLS],
                          respf_d[:, 6912:RESP_COLS])
        nc.sync.dma_start(resp16_sb[:, 6912:RESP_COLS],
                          resp16_d[:, 6912:RESP_COLS])

        # ---------------- static small tiles ----------------
        bias_sb = []
        for s in range(5):
            bt = sb.tile([128, 1], F32, tag=f"bias{s}", name=f"bias{s}")
            nc.vector.memset(bt[:], -CSH[s])
            bias_sb.append(bt)
        eps_sb = sb.tile([128, 1], F32, tag="eps", name="eps_sb")
        nc.vector.memset(eps_sb[:], EPS)

        t_sb = sb.tile([64, QL], F32R, tag="t_sb", name="t_sb")
        p_all = sb.tile([64, 5364], F32R, tag="p_all", name="p_all")
        g_all = sb.tile([128, 128 * NGBLK], BF16, tag="g_all", name="g_all")
        g3 = g_all[:].rearrange("p (k c) -> p k c", c=128)
        stats_sb = sb.tile([128, 325], F32, tag="stats", name="stats_sb")
        mom = sb.tile([128, 20], F32, tag="mom", name="mom")
        scratch = dram.tile([320, QL], BF16, name="scratch")
        xt5 = [sb.tile([64, QL], BF16, tag=f"x{s}", name=f"x{s}")
               for s in range(5)]

        # ---------------- t conv (Act-evacuated) ----------------
        for qc in range(NQC):
            tp = psA.tile([64, QC], F32, tag="sc", name="tp")
            for t in range(2):
                nc.tensor.matmul(
                    tp[:],
                    gwb_sb[:, TWT16_C + 64 * t:TWT16_C + 64 * t + 64],
                    persp_sb[t][:, QC * qc:QC * qc + QC],
                    start=(t == 0), stop=(t == 1))
            nc.scalar.copy(t_sb[:, QC * qc:QC * qc + QC], tp[:])

        # ---------------- p/g conv emitters ----------------
        def emit_p_chunk(s, off, w, on_act):
            pp = psA.tile([64, w], F32, tag="sc", name="pp",
                          padded_shape=[128, QC])
            nch = RCH[s]
            for ci, (rcol, co, cw) in enumerate(nch):
                if s == 0:
                    nc.tensor.matmul(
                        pp[0:64, 0:w],
                        gwb_sb[0:cw, PWT16_C:PWT16_C + 64],
                        resp16_sb[0:cw, rcol + off:rcol + off + w],
                        start=True, stop=True)
                else:
                    nc.tensor.matmul(
                        pp[0:64, 0:w],
                        wbr_sb[0:cw, PWT_C + 64 * (CK0[s] + ci):
                               PWT_C + 64 * (CK0[s] + ci) + 64],
                        respf_sb[0:cw, rcol + off:rcol + off + w],
                        start=(ci == 0), stop=(ci == len(nch) - 1))
            dst = p_all[0:64, P_OFF[s] + off:P_OFF[s] + off + w]
            if on_act:
                nc.scalar.copy(dst, pp[0:64, 0:w])
            else:
                nc.vector.tensor_copy(dst, pp[0:64, 0:w])

        def emit_g_pack(s, chunks, on_act):
            # chunks: list of (j_local_in_pack, k_global, moff, mw)
            n = len(chunks)
            mwmax = max(c[3] for c in chunks)
            gp = psA.tile([128, 64 * n], F32, tag="sc", name="gp",
                          padded_shape=[128, QC])
            nch = RCH[s]
            for j, gk, moff, mw in chunks:
                for ci, (rcol, co, cw) in enumerate(nch):
                    nc.tensor.matmul(
                        gp[0:mw, 64 * j:64 * j + 64],
                        resp16_sb[0:cw, rcol + moff:rcol + moff + mw],
                        gwb_sb[0:cw, 64 * (CK0[s] + ci):
                               64 * (CK0[s] + ci) + 64],
                        start=(ci == 0), stop=(ci == len(nch) - 1))
            k0 = chunks[0][1]
            dst = g3[0:mwmax, k0:k0 + n, 64:128]
            src = gp[0:mwmax, :].rearrange("p (k c) -> p k c", c=64)
            if on_act:
                nc.scalar.copy(dst, src)
            else:
                nc.vector.tensor_copy(dst, src)

        def pg_tasks(s, on_act=False):
            """(p_tasks, g_tasks) emitting scale-s p conv + g conv."""
            ptasks, tasks = [], []
            off = 0
            for w in PCH[s]:
                ptasks.append(lambda s=s, off=off, w=w: emit_p_chunk(
                    s, off, w, on_act))
                off += w
            mts = mtiles(MS[s])
            pack = []
            for k, (moff, mw) in enumerate(mts):
                if mw == 128:
                    pack.append((len(pack), GK0[s] + k, moff, mw))
                    if len(pack) == 6:
                        tasks.append(lambda s=s, p=tuple(pack):
                                     emit_g_pack(s, list(p), on_act))
                        pack = []
                else:
                    if pack:
                        tasks.append(lambda s=s, p=tuple(pack):
                                     emit_g_pack(s, list(p), on_act))
                        pack = []
                    tasks.append(lambda s=s, p=((0, GK0[s] + k, moff, mw),):
                                 emit_g_pack(s, list(p), on_act))
            if pack:
                tasks.append(lambda s=s, p=tuple(pack):
                             emit_g_pack(s, list(p), on_act))
            return ptasks, tasks

        # minimal scale-0 prefix: first p chunk + first g pack (Act evacs),
        # the rest rides the in-loop interleave
        nc.vector.memset(g3[:, :, 0:64], 1.0)   # ones cols (denominator rows)
        p0t, g0t = pg_tasks(0, on_act=True)
        p0t[0]()
        g0t[0]()

        # ---------------- attention ----------------
        pending = deque([p0t[1], p0t[2], g0t[1], p0t[3], p0t[4],
                         g0t[2], p0t[5]])
        for s in range(5):
            while s > 0 and pending:   # prior scales' convs must be done
                pending.popleft()()
            if s == 0:
                p1t, g1t = pg_tasks(1, on_act=False)
                pending.extend(p1t + g1t)
            elif s == 1:
                for s2 in (2, 3, 4):
                    pt, gt = pg_tasks(s2, on_act=False)
                    pending.extend(pt + gt)
            outT = sba.tile([64, QL + 18], BF16, tag="outT", name="outT")
            mts = mtiles(MS[s])
            nk = len(mts)
            ops = [psB.tile([128, QC], F32, tag="op", name=f"opacc{qc}")
                   for qc in range(NQC)]
            ets = {}

            def emit_op(k, s=s, ops=ops, ets=ets, mts=mts, nk=nk):
                moff, mw = mts[k]
                for qc in range(NQC):
                    nc.tensor.matmul(
                        ops[qc][:], g3[0:mw, GK0[s] + k, :],
                        ets[k % 4][0:mw, QC * qc:QC * qc + QC],
                        start=(k == 0), stop=(k == nk - 1))

            for k, (moff, mw) in enumerate(mts):
                scps = []
                for qc in range(NQC):
                    scp = psA.tile([128, QC], F32, tag="sc", name="scp")
                    nc.tensor.matmul(
                        scp[0:mw, :],
                        p_all[:, P_OFF[s] + moff:P_OFF[s] + moff + mw],
                        t_sb[:, QC * qc:QC * qc + QC],
                        start=True, stop=True)
                    scps.append(scp)
                et = sba.tile([128, QL], BF16, tag="et", name="et", bufs=4)
                for qc in range(NQC):
                    nc.scalar.activation(et[0:mw, QC * qc:QC * qc + QC],
                                         scps[qc][0:mw, :], AFT.Exp,
                                         bias=bias_sb[s][0:mw, :])
                ets[k % 4] = et
                if k >= 2:
                    emit_op(k - 2)
                if pending:
                    pending.popleft()()
            for k in range(max(nk - 2, 0), nk):
                emit_op(k)
            # softmax finish straight out of PSUM
            for qc in range(NQC):
                rc = sba.tile([64, QC], F32, tag="rc", name="rc", bufs=3)
                nc.vector.reciprocal_approx_fast(rc[:], ops[qc][0:64, :])
                nc.vector.tensor_tensor(outT[:, QC * qc:QC * qc + QC],
                                        ops[qc][64:128, :], rc[:],
                                        op=AluOpType.mult)
            if DEBUG and s == 0:
                nc.sync.dma_start(dbgo_d, outT[:, 0:QL])
                for qc in range(NQC):
                    dtmp = sba.tile([128, QC], F32, tag="dtmp", name="dtmp",
                                    bufs=3)
                    nc.vector.tensor_copy(dtmp[:], ops[qc][:])
                    nc.sync.dma_start(dbgd_d[:, QC * qc:QC * qc + QC],
                                      dtmp[:])
            # Gram (ones-extended) + scramble-write to DRAM scratch
            nc.vector.memset(outT[:, QL:QL + 18], 1.0)
            ot3 = outT[:].rearrange("p (c j) -> p c j", j=18)
            gm = psB.tile([65, 65], F32, tag="op", name="gm",
                          padded_shape=[128, QC])
            for j in range(18):
                nc.tensor.matmul(gm[:], ot3[:, :, j], ot3[:, :, j],
                                 start=(j == 0), stop=(j == 17))
            nc.vector.tensor_copy(stats_sb[0:65, 65 * s:65 * s + 65], gm[:])
            sc_ap = bass.AP(tensor=scratch[:].tensor,
                            offset=scratch[:].offset + 64 * s * QL,
                            ap=[[18, 64], [QL, 64], [1, 18]])
            nc.sync.dma_start(sc_ap, outT[:, 0:QL])
            nc.sync.dma_start(xt5[s][:], scratch[64 * s:64 * s + 64, :])

        # ------- local z-moments from the local Gram (pre-collective) -------
        # column index g = 5t+s throughout; mom = [sum z^2 (10) | sum z (10)]
        zz = sba.tile([128, 650], F32, tag="zz", name="zz")
        zgs = []
        for t in range(2):
            zg5 = psA.tile([128, 325], F32, tag="sc", name="zg5",
                           padded_shape=[128, QC])
            nc.tensor.matmul(zg5[:],
                             wbf_sb[0:64, ZWT_C + 128 * t:ZWT_C + 128 * t + 128],
                             stats_sb[0:64, :], start=True, stop=True)
            zgs.append(zg5)
        for t in range(2):
            nc.vector.tensor_tensor(zz[:, 325 * t:325 * t + 325], zgs[t][:],
                                    wbf_sb[:, ZW5_C + 325 * t:
                                           ZW5_C + 325 * t + 325],
                                    op=AluOpType.mult)
            nc.vector.tensor_copy(
                mom[:, 10 + 5 * t:15 + 5 * t],
                zgs[t][:].rearrange("p (s m) -> p s m", m=65)[:, :, 64])
        nc.vector.tensor_reduce(
            mom[:, 0:10], zz[:].rearrange("p (g m) -> p g m", m=65),
            mybir.AxisListType.X, AluOpType.add)

        # ---------------- moments AllGather (8x smaller+faster than AR) ----
        arin = dram.tile([128, 20], F32, name="arin")
        arout = dram.tile([128 * NCORES, 20], F32, name="arout")
        nc.sync.dma_start(arin[:], mom[:])
        nc.gpsimd.collective_compute(
            "AllGather", AluOpType.bypass,
            replica_groups=[list(range(NCORES))],
            ins=[arin.opt()], outs=[arout.opt()])

        # keep the PE p-state warm while the collective runs
        warm = psB.tile([128, QC], F32, tag="op", name="warm")
        NWARM = 135
        for i in range(NWARM):
            nc.tensor.matmul(warm[:], resp16_sb[0:128, 0:128],
                             resp16_sb[0:128, 0:QC],
                             start=(i == 0), stop=(i == NWARM - 1))

        agr = sb.tile([128, 20 * NCORES], F32, tag="agr", name="agr")
        nc.sync.dma_start(
            agr[:].rearrange("p (r g) -> p r g", r=NCORES),
            arout[:].rearrange("(r p) g -> p r g", r=NCORES))
        momg = sb.tile([128, 20], F32, tag="momg", name="momg")
        nc.vector.tensor_reduce(
            momg[:], agr[:].rearrange("p (r g) -> p g r", r=NCORES),
            mybir.AxisListType.X, AluOpType.add)

        # ---------------- post-stats (vectorized over scales AND halves) ----
        a10 = sb.tile([128, 10], F32, tag="a10", name="a10")
        bacc2 = sb.tile([128, 2], F32, tag="bacc2", name="bacc2")
        mean10 = sba.tile([128, 10], F32, tag="mean10", name="mean10")
        nc.vector.tensor_scalar_mul(mean10[:], momg[:, 10:20], 1.0 / NPIX)
        m210 = sba.tile([128, 10], F32, tag="m210", name="m210")
        nc.vector.tensor_tensor(m210[:], mean10[:], mean10[:],
                                op=AluOpType.mult)
        var10 = sba.tile([128, 10], F32, tag="var10", name="var10")
        nc.vector.scalar_tensor_tensor(var10[:], momg[:, 0:10], 1.0 / NPIX,
                                       m210[:], op0=AluOpType.mult,
                                       op1=AluOpType.subtract)
        sq10 = sba.tile([128, 10], F32, tag="sq10", name="sq10")
        nc.scalar.activation(sq10[:], var10[:], AFT.Sqrt, bias=eps_sb[:])
        a010 = sba.tile([128, 10], F32, tag="a010", name="a010")
        nc.vector.reciprocal_approx_fast(a010[:], sq10[:])
        nc.vector.tensor_tensor(a10[:], a010[:],
                                wbf_sb[:, GAM_C:GAM_C + 10],
                                op=AluOpType.mult)
        tmb10 = sba.tile([128, 10], F32, tag="tmb10", name="tmb10")
        nc.vector.tensor_tensor(tmb10[:], a10[:], mean10[:],
                                op=AluOpType.mult)
        btm2 = sba.tile([128, 2], F32, tag="btm2", name="btm2")
        nc.vector.tensor_reduce(
            btm2[:], tmb10[:].rearrange("p (t s) -> p t s", s=5),
            mybir.AxisListType.X, AluOpType.add)
        nc.vector.tensor_tensor(bacc2[:], wbf_sb[:, BET_C:BET_C + 2],
                                btm2[:], op=AluOpType.subtract)

        # transpose a10 on PE (vs identity), then broadcast rows via one-hot
        # select matmuls -- no DMA, no gpsimd on the tail
        atp = psA.tile([10, 128], F32, tag="sc", name="atp",
                       padded_shape=[128, QC])
        nc.tensor.matmul(atp[:], a10[:], wbf_sb[:, I_C:I_C + 128],
                         start=True, stop=True)
        a_sbT = sb.tile([10, 128], F32, tag="a_sbT", name="a_sbT")
        nc.vector.tensor_copy(a_sbT[:], atp[0:10, :])

        # wt tiles: W_s = zwt * a_s  (bf16), one [64, CP] tile per scale
        wt5 = [sb.tile([64, CP], BF16, tag=f"wt{s}", name=f"wt{s}")
               for s in range(5)]
        for s in range(5):
            abp = psB.tile([64, CP], F32, tag="op", name="abp",
                           padded_shape=[128, QC])
            for t in range(2):
                r = 5 * t + s
                nc.tensor.matmul(abp[:, 128 * t:128 * t + 128],
                                 wbf_sb[0:10, SEL_C + 64 * r:SEL_C + 64 * r + 64],
                                 a_sbT[:], start=True, stop=True)
            nc.vector.tensor_tensor(wt5[s][:],
                                    wbf_sb[0:64, ZWT_C:ZWT_C + 256],
                                    abp[:], op=AluOpType.mult)

        if DEBUG:
            for s in range(5):
                nc.sync.dma_start(dbgx_d[64 * s:64 * s + 64, :], xt5[s][:])
            nc.sync.dma_start(dbgp_d, p_all[:])
            nc.sync.dma_start(dbgt_d, t_sb[:])
            nc.sync.dma_start(dbgg_d, g_all[:])
            nc.sync.dma_start(dbgs_d[:, 0:325], stats_sb[:])
            nc.sync.dma_start(dbgs_d[:, 325:345], mom[:])
            nc.sync.dma_start(dbgs_d[:, 345:365], momg[:])
            nc.sync.dma_start(dbgs_d[:, 365:375], a10[:])
            nc.sync.dma_start(dbgs_d[:, 375:377], bacc2[:])

        # ---------------- final matmul + unscramble evac + store ----------------
        for t in range(2):
            out_sb = sb.tile([128, QL], BF16, tag=f"outsb{t}", name=f"outsb{t}")
            ov = out_sb[:].rearrange("p (j c) -> p c j", c=64)
            fps = []
            for k in range(5):
                for c in range(4):
                    if k == 0:
                        fps.append(psA.tile([128, 288], F32, tag="sc",
                                            name=f"fp{c}",
                                            padded_shape=[128, QC]))
                    nc.tensor.matmul(fps[c][:],
                                     wt5[k][:, 128 * t:128 * t + 128],
                                     xt5[k][:, 288 * c:288 * c + 288],
                                     start=(k == 0), stop=(k == 4))
            for c in range(4):
                nc.vector.tensor_scalar_add(
                    ov[:, 16 * c:16 * c + 16, :],
                    fps[c][:].rearrange("p (c j) -> p c j", j=18),
                    bacc2[:, t:t + 1])
            nc.sync.dma_start(out_d[128 * t:128 * t + 128, :], out_sb[:])

    nc.compile()
    return nc


def pack_inputs(inputs):
    """Host-side packing shared across cores (weights)."""
    f32 = np.float32
    bf16 = ml_dtypes.bfloat16
    t_w = np.asarray(inputs['t_w'], dtype=f32)
    z_w = np.asarray(inputs['z_w'], dtype=f32)

    wbr = np.zeros((128, WBR_COLS), f32)
    gi = 0
    for s in range(5):
        pw = np.asarray(inputs[f'p{s}_w'], dtype=f32).T  # [C, 64]
        for co in range(0, CR[s], 128):
            w = min(128, CR[s] - co)
            wbr[0:w, PWT_C + 64 * gi:PWT_C + 64 * gi + 64] = pw[co:co + w]
            gi += 1
    twt = t_w.T  # [CP, 64]

    wbf = np.zeros((128, WBF_COLS), f32)
    wbf[0:64, ZWT_C:ZWT_C + 256] = z_w.T          # zwt [64, 256]
    for t in range(2):
        for s in range(5):
            wbf[:, ZW5_C + 325 * t + 65 * s:
                ZW5_C + 325 * t + 65 * s + 64] = z_w[128 * t:128 * t + 128, :]
            wbf[:, GAM_C + 5 * t + s] = np.asarray(
                inputs[f'bn{s}_g'], f32)[128 * t:128 * t + 128]
        wbf[:, BET_C + t] = sum(
            np.asarray(inputs[f'bn{s}_b'], f32)[128 * t:128 * t + 128]
            for s in range(5))
    wbf[:, I_C:I_C + 128] = np.eye(128, dtype=f32)
    for r in range(10):
        wbf[r, SEL_C + 64 * r:SEL_C + 64 * r + 64] = 1.0

    gwb = np.zeros((128, 64 * 31 + 192), bf16)
    gwb[0:64, PWT16_C:PWT16_C + 64] = \
        np.asarray(inputs['p0_w'], f32).T.astype(bf16)
    for t in range(2):
        gwb[:, TWT16_C + 64 * t:TWT16_C + 64 * t + 64] = \
            twt[128 * t:128 * t + 128].astype(bf16)
    gi = 0
    for s in range(5):
        gw = np.asarray(inputs[f'g{s}_w'], dtype=f32).T  # [C, 64]
        for co in range(0, CR[s], 128):
            w = min(128, CR[s] - co)
            gwb[0:w, 64 * gi:64 * gi + 64] = gw[co:co + w].astype(bf16)
            gi += 1
    return wbr, wbf, gwb


def kernel(**inputs):
    f32 = np.float32
    bf16 = ml_dtypes.bfloat16
    persp = np.asarray(inputs['perspective'], dtype=f32)
    if 'nc' not in _CACHED:
        _CACHED['nc'] = build()
    nc = _CACHED['nc']

    wbr, wbf, gwb = pack_inputs(inputs)
    resps = [np.asarray(inputs[f'response{s}'], dtype=f32).reshape(
        4, CR[s], MS[s]) for s in range(5)]

    in_maps = []
    for i in range(4):
        respf = np.zeros((128, RESP_COLS), f32)
        for s in range(5):
            for (rcol, co, w) in RCH[s]:
                respf[0:w, rcol:rcol + MS[s]] = resps[s][i][co:co + w]
        resp16 = respf.astype(bf16)
        for h in range(2):
            cj = (36 * np.arange(64)[:, None] + 18 * h
                  + np.arange(18)[None, :]).ravel()
            m = {
                "persp": np.ascontiguousarray(
                    persp[i].reshape(CP, 2304)[:, cj].astype(bf16)),
                "respf": respf, "resp16": resp16,
                "wbr": wbr, "wbf": wbf, "gwb": gwb,
            }
            in_maps.append(m)
    res = bass_utils.run_bass_kernel_spmd(nc, in_maps,
                                          core_ids=list(range(NCORES)))
    out = np.zeros((4, CP, 2304), np.float32)
    for i in range(4):
        for h in range(2):
            out[i][:, QL * h:QL * h + QL] = \
                res.results[i * 2 + h]["out"].astype(np.float32)
    return out.reshape(4, CP, 48, 48)


if __name__ == "__main__":
    from concourse.timeline_sim import TimelineSim
    nc = build()
    tl = TimelineSim(nc, trace=False)
    print(f"TimelineSim: {tl.simulate():.0f} ns")
